# revision 9
# baseline (speedup 1.0000x reference)
"""STFT (Conv1D-style) Bass kernel for Trainium2, 8 NeuronCores.

Strategy (data-parallel over batch):
  - B=16 signals, 8 cores -> 2 signals per core.
  - Host: reflect-pad, cast to bf16, and lay the padded signal out as
    128-sample blocks transposed to [128, nblocks] (partition = offset
    within block, column = block index). Because HOP=256 = 2*128, frame t
    chunk c (128 samples starting at 256t+128c) is exactly block 2t+c, so
    the stationary matmul operand for an M-tile of frames is a stride-2
    column slice of this layout -- no on-device gather/transpose needed.
  - Host: build the windowed DFT basis Bc[n, 2f] = cos(2pi k n/N)*w[n],
    Bc[n, 2f+1] = -sin(2pi k n/N)*w[n]  ([1024, 1026], bf16), so one
    matmul produces the interleaved (real, imag) output layout directly.
  - Device: out[t, f2] = sum_n frames[t, n] * Bc[n, f2] as 8 accumulating
    K-chunk matmuls (K=128) per (M-tile of 128 frames, N-chunk of <=512).
  - Gather per-core [2, 1876, 1026] f32 outputs -> [16, 1876, 513, 2].
"""

import numpy as np
import ml_dtypes

N_FFT = 1024
HOP = 256
B = 16
T = 480000
F = N_FFT // 2 + 1          # 513
F2 = 2 * F                  # 1026
PAD = N_FFT // 2            # 512
XP_LEN = T + 2 * PAD        # 481024
NBLK = XP_LEN // 128        # 3758
NBLK_PAD = 3760             # padded to a multiple of 16 blocks
NF = (XP_LEN - N_FFT) // HOP + 1   # 1876 frames
NCORES = 8
B_PER_CORE = B // NCORES    # 2
N_SPLIT = [(0, 512), (512, 512), (1024, 2)]

_CACHE = {}


MODE = "fp32r"          # "bf16" | "fp32r"


def _in_dt(mybir):
    return mybir.dt.bfloat16 if MODE == "bf16" else mybir.dt.float32r


def _build_nc(repeat=1):
    import concourse.mybir as mybir
    import concourse.tile as tile
    from concourse import bacc

    idt = _in_dt(mybir)
    nc = bacc.Bacc("TRN2", target_bir_lowering=False, debug=False,
                   num_devices=NCORES)
    xpt = nc.dram_tensor("xpt", [128, B_PER_CORE, NBLK_PAD // 2, 2],
                         idt, kind="ExternalInput")
    basis = nc.dram_tensor("basis", [128, 8, F2], idt,
                           kind="ExternalInput")
    out = nc.dram_tensor("out", [B_PER_CORE, NF, F2], mybir.dt.float32,
                         kind="ExternalOutput")

    with tile.TileContext(nc) as tc:
        with (
            tc.tile_pool(name="sig", bufs=min(repeat, 2)) as sigp,
            tc.tile_pool(name="bas", bufs=min(repeat, 2)) as basp,
            tc.tile_pool(name="outp", bufs=3) as outp,
            tc.tile_pool(name="ps", bufs=2, space="PSUM") as psp,
        ):
            for _rep in range(repeat):
                sig = sigp.tile([128, B_PER_CORE, NBLK_PAD // 2, 2], idt,
                                name="sig", tag="sig")
                bas = basp.tile([128, 8, F2], idt, name="bas", tag="bas")
                # chunked input DMAs so the first matmuls only wait for the
                # slices they read (basis chunk c=0/1 + batch-0 signal first)
                nc.sync.dma_start(bas[:, 0:2], basis[:, 0:2])
                nc.sync.dma_start(sig[:, 0], xpt[:, 0])
                nc.sync.dma_start(bas[:, 2:4], basis[:, 2:4])
                nc.sync.dma_start(bas[:, 4:6], basis[:, 4:6])
                nc.sync.dma_start(bas[:, 6:8], basis[:, 6:8])
                nc.sync.dma_start(sig[:, 1], xpt[:, 1])

                for b in range(B_PER_CORE):
                    for t0 in range(0, NF, 128):
                        M = min(128, NF - t0)
                        pss = [
                            psp.tile([128, w], mybir.dt.float32, tag=f"ps{i}",
                                     name=f"ps{i}")
                            for i, (_, w) in enumerate(N_SPLIT)
                        ]
                        for c in range(8):
                            q, r = divmod(c, 2)
                            lhsT = sig[:, b, t0 + q:t0 + q + M, r]
                            for i, (o, w) in enumerate(N_SPLIT):
                                nc.tensor.matmul(
                                    pss[i][:M, :], lhsT, bas[:, c, o:o + w],
                                    start=(c == 0), stop=(c == 7),
                                )
                        ot = outp.tile([128, F2], mybir.dt.float32)
                        for i, (o, w) in enumerate(N_SPLIT):
                            nc.vector.tensor_copy(ot[:M, o:o + w],
                                                  pss[i][:M, :])
                        nc.sync.dma_start(out[b, t0:t0 + M, :], ot[:M, :])

    nc.compile()
    return nc


def _host_prep(x, window):
    xp = np.pad(x.astype(np.float32), ((0, 0), (PAD, PAD)), mode="reflect")
    xp = np.concatenate(
        [xp, np.zeros((B, NBLK_PAD * 128 - XP_LEN), np.float32)], axis=1)
    # [B, nblk, 128] -> [B, 128, nblk]
    xpt = np.ascontiguousarray(xp.reshape(B, NBLK_PAD, 128).transpose(0, 2, 1))
    np_dt = ml_dtypes.bfloat16 if MODE == "bf16" else np.float32
    xpt_bf = xpt.astype(np_dt)

    # Windowed DFT basis, computed in fp32 to match the reference math.
    k = np.arange(F, dtype=np.float32)[:, None]
    n = np.arange(N_FFT, dtype=np.float32)[None, :]
    ang = np.float32(2.0 * np.pi / N_FFT) * k * n        # [F, N] fp32
    w32 = window.astype(np.float32)
    cosk = np.cos(ang) * w32[None, :]
    sink = -np.sin(ang) * w32[None, :]
    Bc = np.empty((N_FFT, F2), np.float32)
    Bc[:, 0::2] = cosk.T
    Bc[:, 1::2] = sink.T
    basis_sb = np.ascontiguousarray(
        Bc.reshape(8, 128, F2).transpose(1, 0, 2)).astype(np_dt)

    in_maps = []
    for c in range(NCORES):
        xc = xpt_bf[B_PER_CORE * c:B_PER_CORE * (c + 1)]   # [2, 128, 3760]
        xc = np.ascontiguousarray(xc.transpose(1, 0, 2)).reshape(
            128, B_PER_CORE, NBLK_PAD // 2, 2)
        in_maps.append({"xpt": xc, "basis": basis_sb})
    return in_maps


def kernel(x, window):
    from concourse.bass_utils import run_bass_kernel_spmd

    if "nc" not in _CACHE:
        _CACHE["nc"] = _build_nc()
    nc = _CACHE["nc"]

    in_maps = _host_prep(np.asarray(x), np.asarray(window))
    res = run_bass_kernel_spmd(nc, in_maps, core_ids=list(range(NCORES)),
                               trace=False)
    out = np.concatenate([res.results[c]["out"] for c in range(NCORES)],
                         axis=0)                     # [16, 1876, 1026]
    return np.ascontiguousarray(out.reshape(B, NF, F, 2))


# revision 15
# speedup vs baseline: 2.1291x; 2.1291x over previous
"""STFT (Conv1D-style) Bass kernel for Trainium2, 8 NeuronCores.

Strategy (data-parallel over batch):
  - B=16 signals, 8 cores -> 2 signals per core.
  - Host: reflect-pad, cast to bf16, and lay the padded signal out as
    128-sample blocks transposed to [128, nblocks] (partition = offset
    within block, column = block index). Because HOP=256 = 2*128, frame t
    chunk c (128 samples starting at 256t+128c) is exactly block 2t+c, so
    the stationary matmul operand for an M-tile of frames is a stride-2
    column slice of this layout -- no on-device gather/transpose needed.
  - Host: build the windowed DFT basis Bc[n, 2f] = cos(2pi k n/N)*w[n],
    Bc[n, 2f+1] = -sin(2pi k n/N)*w[n]  ([1024, 1026], bf16), so one
    matmul produces the interleaved (real, imag) output layout directly.
  - Device: out[t, f2] = sum_n frames[t, n] * Bc[n, f2] as 8 accumulating
    K-chunk matmuls (K=128) per (M-tile of 128 frames, N-chunk of <=512).
  - Gather per-core [2, 1876, 1026] f32 outputs -> [16, 1876, 513, 2].
"""

import numpy as np
import ml_dtypes

N_FFT = 1024
HOP = 256
B = 16
T = 480000
F = N_FFT // 2 + 1          # 513
F2 = 2 * F                  # 1026
PAD = N_FFT // 2            # 512
XP_LEN = T + 2 * PAD        # 481024
NBLK = XP_LEN // 128        # 3758
NBLK_PAD = 3760             # padded to a multiple of 16 blocks
NF = (XP_LEN - N_FFT) // HOP + 1   # 1876 frames
NCORES = 8
B_PER_CORE = B // NCORES    # 2
# Device computes k=0..511 (1024 interleaved re/im columns); the k=512
# (Nyquist) pair is 0.1% of the FLOPs and would cost 240 tiny N=2 matmuls
# (~30us of unhidden weight loads), so it is done on host BLAS instead.
F2D = 1024
N_SPLIT = [(0, 512), (512, 512)]

_CACHE = {}


MODE = "fp32r"          # "bf16" | "fp32r"


def _in_dt(mybir):
    return mybir.dt.bfloat16 if MODE == "bf16" else mybir.dt.float32r


def _build_nc(repeat=1):
    import concourse.mybir as mybir
    import concourse.tile as tile
    from concourse import bacc

    idt = _in_dt(mybir)
    nc = bacc.Bacc("TRN2", target_bir_lowering=False, debug=False,
                   num_devices=NCORES)
    xpt = nc.dram_tensor("xpt", [128, B_PER_CORE, NBLK_PAD // 2, 2],
                         idt, kind="ExternalInput")
    basis = nc.dram_tensor("basis", [128, 8, F2D], idt,
                           kind="ExternalInput")
    out = nc.dram_tensor("out", [B_PER_CORE, NF, F2D], mybir.dt.float32,
                         kind="ExternalOutput")

    with tile.TileContext(nc) as tc:
        with (
            tc.tile_pool(name="sig", bufs=min(repeat, 2)) as sigp,
            tc.tile_pool(name="bas", bufs=min(repeat, 2)) as basp,
            tc.tile_pool(name="outp", bufs=3) as outp,
            tc.tile_pool(name="ps", bufs=2, space="PSUM") as psp,
        ):
            for _rep in range(repeat):
                sig = sigp.tile([128, B_PER_CORE, NBLK_PAD // 2, 2], idt,
                                name="sig", tag="sig")
                bas = basp.tile([128, 8, F2D], idt, name="bas", tag="bas")
                # chunked input DMAs so the first matmuls only wait for the
                # slices they read (basis chunk c=0/1 + batch-0 signal first)
                nc.sync.dma_start(bas[:, 0:2], basis[:, 0:2])
                nc.sync.dma_start(sig[:, 0], xpt[:, 0])
                nc.sync.dma_start(bas[:, 2:4], basis[:, 2:4])
                nc.sync.dma_start(bas[:, 4:6], basis[:, 4:6])
                nc.sync.dma_start(bas[:, 6:8], basis[:, 6:8])
                nc.sync.dma_start(sig[:, 1], xpt[:, 1])

                for b in range(B_PER_CORE):
                    for t0 in range(0, NF, 128):
                        M = min(128, NF - t0)
                        pss = [
                            psp.tile([128, w], mybir.dt.float32, tag=f"ps{i}",
                                     name=f"ps{i}")
                            for i, (_, w) in enumerate(N_SPLIT)
                        ]
                        for c in range(8):
                            q, r = divmod(c, 2)
                            lhsT = sig[:, b, t0 + q:t0 + q + M, r]
                            for i, (o, w) in enumerate(N_SPLIT):
                                nc.tensor.matmul(
                                    pss[i][:M, :], lhsT, bas[:, c, o:o + w],
                                    start=(c == 0), stop=(c == 7),
                                )
                        ot = outp.tile([128, F2D], mybir.dt.float32)
                        for i, (o, w) in enumerate(N_SPLIT):
                            nc.vector.tensor_copy(ot[:M, o:o + w],
                                                  pss[i][:M, :])
                        nc.sync.dma_start(out[b, t0:t0 + M, :], ot[:M, :])

    nc.compile()
    return nc


def _host_prep(x, window):
    xp = np.pad(x.astype(np.float32), ((0, 0), (PAD, PAD)), mode="reflect")
    xp = np.concatenate(
        [xp, np.zeros((B, NBLK_PAD * 128 - XP_LEN), np.float32)], axis=1)
    # [B, nblk, 128] -> [B, 128, nblk]
    xpt = np.ascontiguousarray(xp.reshape(B, NBLK_PAD, 128).transpose(0, 2, 1))
    np_dt = ml_dtypes.bfloat16 if MODE == "bf16" else np.float32
    xpt_bf = xpt.astype(np_dt)

    # Windowed DFT basis, computed in fp32 to match the reference math.
    k = np.arange(F, dtype=np.float32)[:, None]
    n = np.arange(N_FFT, dtype=np.float32)[None, :]
    ang = np.float32(2.0 * np.pi / N_FFT) * k * n        # [F, N] fp32
    w32 = window.astype(np.float32)
    cosk = np.cos(ang) * w32[None, :]
    sink = -np.sin(ang) * w32[None, :]
    Bc = np.empty((N_FFT, F2), np.float32)
    Bc[:, 0::2] = cosk.T
    Bc[:, 1::2] = sink.T
    basis_sb = np.ascontiguousarray(
        Bc[:, :F2D].reshape(8, 128, F2D).transpose(1, 0, 2)).astype(np_dt)

    # k=512 (Nyquist) re/im pair on host BLAS (fp32, exact-grade).
    ny_basis = np.ascontiguousarray(Bc[:, F2D:])          # [1024, 2]
    nyq = np.empty((B, NF, 2), np.float32)
    for b in range(B):
        frames = np.lib.stride_tricks.as_strided(
            xp[b], (NF, N_FFT), (HOP * 4, 4))
        nyq[b] = frames @ ny_basis

    in_maps = []
    for c in range(NCORES):
        xc = xpt_bf[B_PER_CORE * c:B_PER_CORE * (c + 1)]   # [2, 128, 3760]
        xc = np.ascontiguousarray(xc.transpose(1, 0, 2)).reshape(
            128, B_PER_CORE, NBLK_PAD // 2, 2)
        in_maps.append({"xpt": xc, "basis": basis_sb})
    return in_maps, nyq


def kernel(x, window):
    from concourse.bass_utils import run_bass_kernel_spmd

    if "nc" not in _CACHE:
        _CACHE["nc"] = _build_nc()
    nc = _CACHE["nc"]

    in_maps, nyq = _host_prep(np.asarray(x), np.asarray(window))
    res = run_bass_kernel_spmd(nc, in_maps, core_ids=list(range(NCORES)),
                               trace=False)
    dev = np.concatenate([res.results[c]["out"] for c in range(NCORES)],
                         axis=0)                     # [16, 1876, 1024]
    out = np.empty((B, NF, F, 2), np.float32)
    out[:, :, :F - 1, :] = dev.reshape(B, NF, F - 1, 2)
    out[:, :, F - 1, :] = nyq
    return out
